# revision 20
# baseline (speedup 1.0000x reference)
"""Trainium2 Bass kernel for AttentionConvFull (local 5x5 window attention
with per-channel softmax, grouped 1x1 conv projections).

Sharding: 8 cores = batch(4) x H-halves(2). Each core gets a 32-row halo'd,
zero-padded slice of x, pre-transposed on host to channel-major [256, 32*60].
No collectives needed.

V7 dataflow per core (2 channel-chunks of 128 partitions each):
  - j-loop in dj-major order, batches of 5 j's sharing one dj (one source
    map, one shift parity).
  - kr_j = k_j + rel_j: mostly ACT (Identity + per-partition bias); DVE
    tensor_scalar (4x) for the first batch so the pipeline fills fast.
  - t = kr * q: ONE tensor_tensor per batch (q broadcast over the 5-j dim
    with a stride-0 AP), 2x mode.
  - exp: most batches use a Schraudolph bit-trick on DVE: one dual-op
    tensor_scalar e16 = i16(t*A + B) (fp32 internal, 4x mode) whose int16
    bits reinterpreted as bf16 approximate exp(t) to ~1-2%; softmax
    normalization keeps the end-to-end error ~1e-2 (host-simulated).
    A few batches keep exact ACT exp to balance engine load.
  - w_j = e_j * v_j: per-j DVE TT (2x).
  - den += e_j, num += w_j: identity-matmul PSUM accumulation on PE.
  - Flat software pipeline across (chunk, batch); chunk 1's projections are
    emitted after chunk 0's first batch so the ACT FIFO doesn't delay the
    first exp; per-half epilogue with split output DMA.
  - GPSIMD unused: its tensor_scalar is ~24us/pass and its tensor_tensor
    steals the DVE-shared SBUF port (measured 4x DVE TT slowdown).
  - bf16 output, host upcasts.
"""

import numpy as np
import ml_dtypes

import concourse.bass as bass
import concourse.tile as tile
from concourse import bacc, mybir
from concourse.bass_utils import run_bass_kernel_spmd

F32 = mybir.dt.float32
BF16 = mybir.dt.bfloat16
I16 = mybir.dt.int16

K = 5
G = 8
B, H, W, C = 4, 56, 56, 256
Cg = C // G            # 32
P = K // 2             # 2
HS = H // 2            # 28 output rows per shard
MR = HS + 2 * P        # 32 map rows
MC = W + 2 * P         # 60 map cols
SP = MR * MC           # 1920 map spatial
OP = HS * W            # 1568 output spatial per shard
NCH = 2                # channel chunks of 128 partitions
NCORES = 8
HALF = OP // 2         # 784: PSUM accumulate tile half-size

# Schraudolph constants for bf16: e^t ~= bitcast_bf16(i16(A*t + B))
SCH_A = 128.0 / float(np.log(2.0))   # 184.664
SCH_B = 16250.0                      # 127*128 minus centering correction

# global batch indices (c*5 + dj) that keep exact ACT exp (engine balance;
# placed early-mid in each chunk so ACT drains before the chunk tail)
EXACT_EXP = {1, 2, 5, 6, 7}


def _dedup_ldweights(nc):
    """Remove redundant PE weight reloads: consecutive InstLdweights that
    load the same stationary operand with no sync info."""
    removed = 0
    for blk in nc.main_func.blocks:
        last_sig = None
        keep = []
        for inst in blk.instructions:
            if isinstance(inst, mybir.InstLdweights):
                sig = " ".join(a.concise() for a in inst.ins)
                si = inst.sync_info
                clean = si is None or (
                    len(si.on_wait) == 0 and len(si.on_update) == 0
                )
                if sig == last_sig and clean:
                    removed += 1
                    continue
                last_sig = sig
            elif isinstance(inst, mybir.InstMatmult):
                if len(inst.ins) > 1:
                    wsig = inst.ins[1].concise()
                    if wsig != last_sig:
                        last_sig = wsig
            keep.append(inst)
        blk.instructions[:] = keep
    return removed


def build_nc():
    nc = bacc.Bacc(
        "TRN2", target_bir_lowering=False, debug=False, num_devices=NCORES
    )

    xt_d = nc.dram_tensor("xt", [NCH, 128, SP], BF16, kind="ExternalInput").ap()
    wq_d = nc.dram_tensor("wqb", [NCH, 128, 128], BF16, kind="ExternalInput").ap()
    wk_d = nc.dram_tensor("wkb", [NCH, 128, 128], BF16, kind="ExternalInput").ap()
    wv_d = nc.dram_tensor("wvb", [NCH, 128, 128], BF16, kind="ExternalInput").ap()
    rel_d = nc.dram_tensor("relb", [NCH, 128, K * K], F32, kind="ExternalInput").ap()
    qe_d = nc.dram_tensor("qeb", [NCH, 128, 1], F32, kind="ExternalInput").ap()
    id_d = nc.dram_tensor("idn", [128, 128], BF16, kind="ExternalInput").ap()
    out_d = nc.dram_tensor("out", [NCH, 128, OP], BF16, kind="ExternalOutput").ap()

    JLIST = [(di, dj) for dj in range(K) for di in range(K)]

    with tile.TileContext(nc) as tc:
        with (
            tc.tile_pool(name="consts", bufs=1) as consts,
            tc.tile_pool(name="weights", bufs=2) as wpool,
            tc.tile_pool(name="xin", bufs=2) as xpool,
            tc.tile_pool(name="maps", bufs=2) as mpool,
            tc.tile_pool(name="jwork", bufs=4) as jpool,
            tc.tile_pool(name="qwork", bufs=2) as qpool,
            tc.tile_pool(name="krw", bufs=2) as krpool,
            tc.tile_pool(name="epi", bufs=2) as epool,
            tc.tile_pool(name="acc", bufs=4, space=bass.MemorySpace.PSUM) as psum,
        ):
            ident = consts.tile([128, 128], BF16, tag="ident")
            nc.sync.dma_start(ident[:], id_d)

            kmaps, komaps = [None] * NCH, [None] * NCH
            vmaps, vomaps = [None] * NCH, [None] * NCH
            qflats, rels = [None] * NCH, [None] * NCH

            def emit_proj(c):
                x_sb = xpool.tile([128, SP], BF16, tag="x")
                hsp = SP // 2
                nc.sync.dma_start(x_sb[:, :hsp], xt_d[c][:, :hsp])
                nc.sync.dma_start(x_sb[:, hsp:], xt_d[c][:, hsp:])

                wts = {}
                for nm, d in (("wq", wq_d), ("wk", wk_d), ("wv", wv_d)):
                    t = wpool.tile([128, 128], BF16, tag=nm, name=f"{nm}{c}")
                    nc.sync.dma_start(t[:], d[c])
                    wts[nm] = t
                rel_sb = wpool.tile([128, K * K], F32, tag="rel", name=f"rel{c}")
                nc.sync.dma_start(rel_sb[:], rel_d[c])
                qe_sb = wpool.tile([128, 1], F32, tag="qe", name=f"qe{c}")
                nc.sync.dma_start(qe_sb[:], qe_d[c])
                rels[c] = rel_sb

                k_bf = mpool.tile([128, SP], BF16, tag="k", name=f"k{c}")
                v_bf = mpool.tile([128, SP], BF16, tag="v", name=f"v{c}")
                qf = mpool.tile([128, OP], BF16, tag="qf", name=f"qf{c}")

                # k first then q (j-loop head deps), then v (needed a bit
                # later by the first w-mult); weight-major ldweights
                NS = 2
                SL = SP // NS  # 960
                for nm in ("wk", "wq", "wv"):
                    for s in range(NS):
                        lo = s * SL
                        rhs = x_sb[:, lo : lo + SL]
                        ps = psum.tile(
                            [128, SL], F32, tag="acc", name=f"pp{c}{s}{nm}"
                        )
                        for mlo, mn in ((0, 512), (512, SL - 512)):
                            nc.tensor.matmul(
                                ps[:, mlo : mlo + mn],
                                wts[nm][:],
                                rhs[:, mlo : mlo + mn],
                                start=True,
                                stop=True,
                            )
                        if nm == "wq":
                            r0 = max(P, 16 * s)
                            r1 = min(MR - P, 16 * (s + 1))
                            src = ps[:].rearrange("p (h w) -> p h w", h=16)[
                                :, r0 - 16 * s : r1 - 16 * s, P : P + W
                            ]
                            dst = qf[:].rearrange("p (h w) -> p h w", h=HS)[
                                :, r0 - P : r1 - P, :
                            ]
                            nc.scalar.activation(
                                dst,
                                src,
                                mybir.ActivationFunctionType.Identity,
                                bias=qe_sb[:],
                            )
                        elif nm == "wk":
                            nc.scalar.copy(k_bf[:, lo : lo + SL], ps[:])
                        else:
                            nc.scalar.copy(v_bf[:, lo : lo + SL], ps[:])

                # 1-elem-shifted copies: odd-dj reads stay 4B-aligned
                k_od = mpool.tile([128, SP], BF16, tag="ko", name=f"ko{c}")
                v_od = mpool.tile([128, SP], BF16, tag="vo", name=f"vo{c}")
                nc.sync.dma_start(k_od[:, : SP - 1], k_bf[:, 1:])
                nc.sync.dma_start(v_od[:, : SP - 1], v_bf[:, 1:])

                kmaps[c], komaps[c] = k_bf, k_od
                vmaps[c], vomaps[c] = v_bf, v_od
                qflats[c] = qf

            dens, nums = {}, {}

            def emit_head(c, pr, pos, gbatch):
                rel_sb, qf = rels[c], qflats[c]
                npr = len(pr)
                t2 = qpool.tile([128, npr * OP], BF16, tag="t2", name=f"t{c}{pos}")
                kr_t = krpool.tile(
                    [128, npr * OP], BF16, tag="kr", name=f"kr{c}{pos}"
                )
                vviews = []
                for i, (di, dj) in enumerate(pr):
                    j = di * K + dj
                    if dj % 2 == 0:
                        ksrc, vsrc, dje = kmaps[c], vmaps[c], dj
                    else:
                        ksrc, vsrc, dje = komaps[c], vomaps[c], dj - 1
                    k3 = ksrc[:].rearrange("p (h w) -> p h w", h=MR)
                    v3 = vsrc[:].rearrange("p (h w) -> p h w", h=MR)
                    kv = k3[:, di : di + HS, dje : dje + W]
                    vviews.append(v3[:, di : di + HS, dje : dje + W])
                    kr3 = kr_t[:, i * OP : (i + 1) * OP].rearrange(
                        "p (h w) -> p h w", h=HS
                    )
                    if gbatch <= 1:
                        # head: keep early batches off the busy ACT FIFO
                        # (their kr would queue behind chunk-1's projection
                        # copies and stall the DVE chain)
                        nc.vector.tensor_scalar(
                            kr3,
                            kv,
                            rel_sb[:, j : j + 1],
                            None,
                            mybir.AluOpType.add,
                        )
                    else:
                        nc.scalar.activation(
                            kr3,
                            kv,
                            mybir.ActivationFunctionType.Identity,
                            bias=rel_sb[:, j : j + 1],
                        )
                qb = qf[:].unsqueeze(1).broadcast_to([128, npr, OP])
                nc.vector.tensor_tensor(
                    t2[:].rearrange("p (s f) -> p s f", s=npr),
                    kr_t[:].rearrange("p (s f) -> p s f", s=npr),
                    qb,
                    mybir.AluOpType.mult,
                )
                return (c, pr, pos, gbatch, t2, vviews)

            def emit_tail(staged):
                c, pr, pos, gbatch, t2, vviews = staged
                den, num = dens[c], nums[c]
                npr = len(pr)
                if gbatch in EXACT_EXP:
                    e2 = qpool.tile(
                        [128, npr * OP], BF16, tag="e2", name=f"e{c}{pos}"
                    )
                    nc.scalar.activation(
                        e2[:], t2[:], mybir.ActivationFunctionType.Exp
                    )
                    e_bf = e2[:]
                else:
                    # Schraudolph: e16 = i16(t*A + B); bits are bf16 exp(t)
                    e16 = qpool.tile(
                        [128, npr * OP], I16, tag="e2", name=f"e{c}{pos}"
                    )
                    nc.vector.tensor_scalar(
                        e16[:],
                        t2[:],
                        SCH_A,
                        SCH_B,
                        mybir.AluOpType.mult,
                        mybir.AluOpType.add,
                    )
                    e_bf = e16[:].bitcast(BF16)
                for i, (di, dj) in enumerate(pr):
                    eflat = e_bf[:, i * OP : (i + 1) * OP]
                    e3 = eflat.rearrange("p (h w) -> p h w", h=HS)
                    w_t = jpool.tile(
                        [128, OP], BF16, tag="w", name=f"w{c}{pos + i}"
                    )
                    w3 = w_t[:].rearrange("p (h w) -> p h w", h=HS)
                    nc.vector.tensor_tensor(
                        w3, e3, vviews[i], mybir.AluOpType.mult
                    )
                    st = pos + i == 0
                    sp = pos + i == K * K - 1
                    for acc, src_t in ((den, eflat), (num, w_t[:])):
                        for h in range(2):
                            base = h * HALF
                            for lo, n in ((0, 512), (512, HALF - 512)):
                                nc.tensor.matmul(
                                    acc[h][:, lo : lo + n],
                                    ident[:],
                                    src_t[:, base + lo : base + lo + n],
                                    start=st,
                                    stop=sp,
                                )

            def emit_epilogue(c):
                den, num = dens[c], nums[c]
                out_sb = epool.tile([128, OP], BF16, tag="osb", name=f"osb{c}")
                for h in range(2):
                    base = h * HALF
                    rden = epool.tile(
                        [128, HALF], F32, tag="rden", name=f"rd{c}{h}"
                    )
                    nc.vector.reciprocal_approx_fast(rden[:], den[h][:])
                    nc.vector.tensor_tensor(
                        out_sb[:, base : base + HALF],
                        num[h][:],
                        rden[:],
                        mybir.AluOpType.mult,
                    )
                    nc.sync.dma_start(
                        out_d[c][:, base : base + HALF],
                        out_sb[:, base : base + HALF],
                    )

            # ---- emission schedule ----
            emit_proj(0)
            worklist = []
            for c in range(NCH):
                for b in range(K):
                    worklist.append((c, JLIST[b * K : (b + 1) * K], b * K, c * K + b))

            staged = None
            for widx, (c, pr, pos, gbatch) in enumerate(worklist):
                if pos == 0:
                    dens[c] = [
                        psum.tile([128, HALF], F32, tag="acc", name=f"den{c}{h}")
                        for h in range(2)
                    ]
                    nums[c] = [
                        psum.tile([128, HALF], F32, tag="acc", name=f"num{c}{h}")
                        for h in range(2)
                    ]
                head = emit_head(c, pr, pos, gbatch)
                if widx == 1:
                    emit_proj(1)
                if staged is not None:
                    emit_tail(staged)
                    if staged[2] == K * K - K and staged[0] != c:
                        emit_epilogue(staged[0])
                staged = head
            emit_tail(staged)
            emit_epilogue(staged[0])

    nc.compile()
    _dedup_ldweights(nc)
    return nc


def _block_diag_weights(w):
    """w: (G, Cg_out, Cg_in) -> lhsT layout [NCH, 128, 128] where
    lhsT[c, ci, co] = w[g, co%32, ci%32] for matching 32-blocks."""
    out = np.zeros((NCH, 128, 128), np.float32)
    for c in range(NCH):
        for g4 in range(4):
            g = c * 4 + g4
            blk = w[g]  # (Cg_out, Cg_in)
            out[c, g4 * 32 : (g4 + 1) * 32, g4 * 32 : (g4 + 1) * 32] = blk.T
    return out


_NC_CACHE = {}


def _make_in_maps(inputs):
    x = np.asarray(inputs["x"], np.float32)
    wq = np.asarray(inputs["wq"], np.float32)
    wk = np.asarray(inputs["wk"], np.float32)
    wv = np.asarray(inputs["wv"], np.float32)
    rel_emb = np.asarray(inputs["rel_emb"], np.float32)
    q_emb = np.asarray(inputs["q_emb"], np.float32)

    bf = ml_dtypes.bfloat16
    wqb = _block_diag_weights(wq).astype(bf)
    wkb = _block_diag_weights(wk).astype(bf)
    wvb = _block_diag_weights(wv).astype(bf)
    relb = np.ascontiguousarray(
        rel_emb.reshape(G, Cg, K * K).reshape(NCH, 128, K * K)
    )
    qeb = np.ascontiguousarray(q_emb.reshape(NCH, 128, 1))
    idn = np.eye(128, dtype=bf)

    xp = np.pad(x, ((0, 0), (P, P), (P, P), (0, 0)))  # (B, 60, 60, C)

    in_maps = []
    for core in range(NCORES):
        b, half = divmod(core, 2)
        sh = xp[b, HS * half : HS * half + MR]         # (32, 60, C)
        xt = np.ascontiguousarray(sh.reshape(SP, C).T).reshape(NCH, 128, SP)
        in_maps.append(
            {
                "xt": xt.astype(bf),
                "wqb": wqb,
                "wkb": wkb,
                "wvb": wvb,
                "relb": relb,
                "qeb": qeb,
                "idn": idn,
            }
        )
    return in_maps


def kernel(**inputs):
    in_maps = _make_in_maps(inputs)

    if "nc" not in _NC_CACHE:
        _NC_CACHE["nc"] = build_nc()
    nc = _NC_CACHE["nc"]

    res = run_bass_kernel_spmd(nc, in_maps, core_ids=list(range(NCORES)))

    out = np.empty((B, H, W, C), np.float32)
    for core in range(NCORES):
        b, half = divmod(core, 2)
        o = np.asarray(res.results[core]["out"]).astype(np.float32)
        o = o.reshape(C, HS, W)
        out[b, HS * half : HS * half + HS] = o.transpose(1, 2, 0)
    return out


# revision 21
# speedup vs baseline: 1.2014x; 1.2014x over previous
"""Trainium2 Bass kernel for AttentionConvFull (local 5x5 window attention
with per-channel softmax, grouped 1x1 conv projections).

Sharding: 8 cores = batch(4) x H-halves(2). Each core gets a 32-row halo'd,
zero-padded slice of x, pre-transposed on host to channel-major [256, 32*60].
No collectives needed.

V7 dataflow per core (2 channel-chunks of 128 partitions each):
  - j-loop in dj-major order, batches of 5 j's sharing one dj (one source
    map, one shift parity).
  - kr_j = k_j + rel_j: mostly ACT (Identity + per-partition bias); DVE
    tensor_scalar (4x) for the first batch so the pipeline fills fast.
  - t = kr * q: ONE tensor_tensor per batch (q broadcast over the 5-j dim
    with a stride-0 AP), 2x mode.
  - exp: most batches use a Schraudolph bit-trick on DVE: one dual-op
    tensor_scalar e16 = i16(t*A + B) (fp32 internal, 4x mode) whose int16
    bits reinterpreted as bf16 approximate exp(t) to ~1-2%; softmax
    normalization keeps the end-to-end error ~1e-2 (host-simulated).
    A few batches keep exact ACT exp to balance engine load.
  - w_j = e_j * v_j: per-j DVE TT (2x).
  - den += e_j, num += w_j: identity-matmul PSUM accumulation on PE.
  - Flat software pipeline across (chunk, batch); chunk 1's projections are
    emitted after chunk 0's first batch so the ACT FIFO doesn't delay the
    first exp; per-half epilogue with split output DMA.
  - GPSIMD unused: its tensor_scalar is ~24us/pass and its tensor_tensor
    steals the DVE-shared SBUF port (measured 4x DVE TT slowdown).
  - bf16 output, host upcasts.
"""

import numpy as np
import ml_dtypes

import concourse.bass as bass
import concourse.tile as tile
from concourse import bacc, mybir
from concourse.bass_utils import run_bass_kernel_spmd

F32 = mybir.dt.float32
BF16 = mybir.dt.bfloat16
I16 = mybir.dt.int16

K = 5
G = 8
B, H, W, C = 4, 56, 56, 256
Cg = C // G            # 32
P = K // 2             # 2
HS = H // 2            # 28 output rows per shard
MR = HS + 2 * P        # 32 map rows
MC = W + 2 * P         # 60 map cols
SP = MR * MC           # 1920 map spatial
OP = HS * W            # 1568 output spatial per shard
NCH = 2                # channel chunks of 128 partitions
NCORES = 8
HALF = OP // 2         # 784: PSUM accumulate tile half-size

# Schraudolph constants for bf16: e^t ~= bitcast_bf16(i16(A*t + B))
SCH_A = 128.0 / float(np.log(2.0))   # 184.664
SCH_B = 16250.0                      # 127*128 minus centering correction

# global batch indices (c*5 + dj) that keep exact ACT exp (engine balance;
# placed early-mid in each chunk so ACT drains before the chunk tail)
EXACT_EXP = {1, 2, 5, 6, 7}


def _dedup_ldweights(nc):
    """Remove redundant PE weight reloads: consecutive InstLdweights that
    load the same stationary operand with no sync info."""
    removed = 0
    for blk in nc.main_func.blocks:
        last_sig = None
        keep = []
        for inst in blk.instructions:
            if isinstance(inst, mybir.InstLdweights):
                sig = " ".join(a.concise() for a in inst.ins)
                si = inst.sync_info
                clean = si is None or (
                    len(si.on_wait) == 0 and len(si.on_update) == 0
                )
                if sig == last_sig and clean:
                    removed += 1
                    continue
                last_sig = sig
            elif isinstance(inst, mybir.InstMatmult):
                if len(inst.ins) > 1:
                    wsig = inst.ins[1].concise()
                    if wsig != last_sig:
                        last_sig = wsig
            keep.append(inst)
        blk.instructions[:] = keep
    return removed


def build_nc():
    nc = bacc.Bacc(
        "TRN2", target_bir_lowering=False, debug=False, num_devices=NCORES
    )

    xt_d = nc.dram_tensor("xt", [NCH, 128, SP], BF16, kind="ExternalInput").ap()
    wq_d = nc.dram_tensor("wqb", [NCH, 128, 128], BF16, kind="ExternalInput").ap()
    wk_d = nc.dram_tensor("wkb", [NCH, 128, 128], BF16, kind="ExternalInput").ap()
    wv_d = nc.dram_tensor("wvb", [NCH, 128, 128], BF16, kind="ExternalInput").ap()
    rel_d = nc.dram_tensor("relb", [NCH, 128, K * K], F32, kind="ExternalInput").ap()
    qe_d = nc.dram_tensor("qeb", [NCH, 128, 1], F32, kind="ExternalInput").ap()
    id_d = nc.dram_tensor("idn", [128, 128], BF16, kind="ExternalInput").ap()
    out_d = nc.dram_tensor("out", [NCH, 128, OP], BF16, kind="ExternalOutput").ap()

    JLIST = [(di, dj) for dj in range(K) for di in range(K)]

    with tile.TileContext(nc) as tc:
        with (
            tc.tile_pool(name="consts", bufs=1) as consts,
            tc.tile_pool(name="weights", bufs=2) as wpool,
            tc.tile_pool(name="xin", bufs=2) as xpool,
            tc.tile_pool(name="maps", bufs=2) as mpool,
            tc.tile_pool(name="jwork", bufs=6) as jpool,
            tc.tile_pool(name="qwork", bufs=2) as qpool,
            tc.tile_pool(name="krw", bufs=2) as krpool,
            tc.tile_pool(name="epi", bufs=2) as epool,
            tc.tile_pool(name="acc", bufs=4, space=bass.MemorySpace.PSUM) as psum,
        ):
            ident = consts.tile([128, 128], BF16, tag="ident")
            nc.sync.dma_start(ident[:], id_d)

            kmaps, komaps = [None] * NCH, [None] * NCH
            vmaps, vomaps = [None] * NCH, [None] * NCH
            qflats, rels = [None] * NCH, [None] * NCH

            def emit_proj(c):
                x_sb = xpool.tile([128, SP], BF16, tag="x")
                hsp = SP // 2
                nc.sync.dma_start(x_sb[:, :hsp], xt_d[c][:, :hsp])
                nc.sync.dma_start(x_sb[:, hsp:], xt_d[c][:, hsp:])

                wts = {}
                for nm, d in (("wq", wq_d), ("wk", wk_d), ("wv", wv_d)):
                    t = wpool.tile([128, 128], BF16, tag=nm, name=f"{nm}{c}")
                    nc.sync.dma_start(t[:], d[c])
                    wts[nm] = t
                rel_sb = wpool.tile([128, K * K], F32, tag="rel", name=f"rel{c}")
                nc.sync.dma_start(rel_sb[:], rel_d[c])
                qe_sb = wpool.tile([128, 1], F32, tag="qe", name=f"qe{c}")
                nc.sync.dma_start(qe_sb[:], qe_d[c])
                rels[c] = rel_sb

                k_bf = mpool.tile([128, SP], BF16, tag="k", name=f"k{c}")
                v_bf = mpool.tile([128, SP], BF16, tag="v", name=f"v{c}")
                qf = mpool.tile([128, OP], BF16, tag="qf", name=f"qf{c}")

                # k first then q (j-loop head deps), then v (needed a bit
                # later by the first w-mult); weight-major ldweights
                NS = 2
                SL = SP // NS  # 960
                for nm in ("wk", "wq", "wv"):
                    for s in range(NS):
                        lo = s * SL
                        rhs = x_sb[:, lo : lo + SL]
                        ps = psum.tile(
                            [128, SL], F32, tag="acc", name=f"pp{c}{s}{nm}"
                        )
                        for mlo, mn in ((0, 512), (512, SL - 512)):
                            nc.tensor.matmul(
                                ps[:, mlo : mlo + mn],
                                wts[nm][:],
                                rhs[:, mlo : mlo + mn],
                                start=True,
                                stop=True,
                            )
                        if nm == "wq":
                            r0 = max(P, 16 * s)
                            r1 = min(MR - P, 16 * (s + 1))
                            src = ps[:].rearrange("p (h w) -> p h w", h=16)[
                                :, r0 - 16 * s : r1 - 16 * s, P : P + W
                            ]
                            dst = qf[:].rearrange("p (h w) -> p h w", h=HS)[
                                :, r0 - P : r1 - P, :
                            ]
                            nc.scalar.activation(
                                dst,
                                src,
                                mybir.ActivationFunctionType.Identity,
                                bias=qe_sb[:],
                            )
                        elif nm == "wk":
                            nc.scalar.copy(k_bf[:, lo : lo + SL], ps[:])
                        else:
                            nc.scalar.copy(v_bf[:, lo : lo + SL], ps[:])

                # 1-elem-shifted copies: odd-dj reads stay 4B-aligned
                k_od = mpool.tile([128, SP], BF16, tag="ko", name=f"ko{c}")
                v_od = mpool.tile([128, SP], BF16, tag="vo", name=f"vo{c}")
                nc.sync.dma_start(k_od[:, : SP - 1], k_bf[:, 1:])
                nc.sync.dma_start(v_od[:, : SP - 1], v_bf[:, 1:])

                kmaps[c], komaps[c] = k_bf, k_od
                vmaps[c], vomaps[c] = v_bf, v_od
                qflats[c] = qf

            dens, nums = {}, {}

            def emit_head(c, pr, pos, gbatch):
                rel_sb, qf = rels[c], qflats[c]
                npr = len(pr)
                t2 = qpool.tile([128, npr * OP], BF16, tag="t2", name=f"t{c}{pos}")
                kr_t = krpool.tile(
                    [128, npr * OP], BF16, tag="kr", name=f"kr{c}{pos}"
                )
                vviews = []
                for i, (di, dj) in enumerate(pr):
                    j = di * K + dj
                    if dj % 2 == 0:
                        ksrc, vsrc, dje = kmaps[c], vmaps[c], dj
                    else:
                        ksrc, vsrc, dje = komaps[c], vomaps[c], dj - 1
                    k3 = ksrc[:].rearrange("p (h w) -> p h w", h=MR)
                    v3 = vsrc[:].rearrange("p (h w) -> p h w", h=MR)
                    kv = k3[:, di : di + HS, dje : dje + W]
                    vviews.append(v3[:, di : di + HS, dje : dje + W])
                    kr3 = kr_t[:, i * OP : (i + 1) * OP].rearrange(
                        "p (h w) -> p h w", h=HS
                    )
                    if gbatch <= 1:
                        # head: keep early batches off the busy ACT FIFO
                        # (their kr would queue behind chunk-1's projection
                        # copies and stall the DVE chain)
                        nc.vector.tensor_scalar(
                            kr3,
                            kv,
                            rel_sb[:, j : j + 1],
                            None,
                            mybir.AluOpType.add,
                        )
                    else:
                        nc.scalar.activation(
                            kr3,
                            kv,
                            mybir.ActivationFunctionType.Identity,
                            bias=rel_sb[:, j : j + 1],
                        )
                qb = qf[:].unsqueeze(1).broadcast_to([128, npr, OP])
                nc.vector.tensor_tensor(
                    t2[:].rearrange("p (s f) -> p s f", s=npr),
                    kr_t[:].rearrange("p (s f) -> p s f", s=npr),
                    qb,
                    mybir.AluOpType.mult,
                )
                return (c, pr, pos, gbatch, t2, vviews)

            def emit_tail(staged):
                c, pr, pos, gbatch, t2, vviews = staged
                den, num = dens[c], nums[c]
                npr = len(pr)
                if gbatch in EXACT_EXP:
                    e2 = qpool.tile(
                        [128, npr * OP], BF16, tag="e2", name=f"e{c}{pos}"
                    )
                    nc.scalar.activation(
                        e2[:], t2[:], mybir.ActivationFunctionType.Exp
                    )
                    e_bf = e2[:]
                else:
                    # Schraudolph: e16 = i16(t*A + B); bits are bf16 exp(t)
                    e16 = qpool.tile(
                        [128, npr * OP], I16, tag="e2", name=f"e{c}{pos}"
                    )
                    nc.vector.tensor_scalar(
                        e16[:],
                        t2[:],
                        SCH_A,
                        SCH_B,
                        mybir.AluOpType.mult,
                        mybir.AluOpType.add,
                    )
                    e_bf = e16[:].bitcast(BF16)
                # w-mults first (DVE), then all den MMs (inputs already
                # ready -> guaranteed PE burst), then all num MMs: avoids PE
                # head-of-line waits on each w_j and keeps HAM warm
                wts_ = []
                for i, (di, dj) in enumerate(pr):
                    eflat = e_bf[:, i * OP : (i + 1) * OP]
                    e3 = eflat.rearrange("p (h w) -> p h w", h=HS)
                    w_t = jpool.tile(
                        [128, OP], BF16, tag="w", name=f"w{c}{pos + i}"
                    )
                    w3 = w_t[:].rearrange("p (h w) -> p h w", h=HS)
                    nc.vector.tensor_tensor(
                        w3, e3, vviews[i], mybir.AluOpType.mult
                    )
                    wts_.append(w_t)
                for acc, srcs in (
                    (den, [e_bf[:, i * OP : (i + 1) * OP] for i in range(npr)]),
                    (num, [w[:] for w in wts_]),
                ):
                    for i in range(npr):
                        st = pos + i == 0
                        sp = pos + i == K * K - 1
                        for h in range(2):
                            base = h * HALF
                            for lo, n in ((0, 512), (512, HALF - 512)):
                                nc.tensor.matmul(
                                    acc[h][:, lo : lo + n],
                                    ident[:],
                                    srcs[i][:, base + lo : base + lo + n],
                                    start=st,
                                    stop=sp,
                                )

            def emit_epilogue(c):
                den, num = dens[c], nums[c]
                out_sb = epool.tile([128, OP], BF16, tag="osb", name=f"osb{c}")
                for h in range(2):
                    base = h * HALF
                    rden = epool.tile(
                        [128, HALF], F32, tag="rden", name=f"rd{c}{h}"
                    )
                    nc.vector.reciprocal_approx_fast(rden[:], den[h][:])
                    nc.vector.tensor_tensor(
                        out_sb[:, base : base + HALF],
                        num[h][:],
                        rden[:],
                        mybir.AluOpType.mult,
                    )
                    nc.sync.dma_start(
                        out_d[c][:, base : base + HALF],
                        out_sb[:, base : base + HALF],
                    )

            # ---- emission schedule ----
            emit_proj(0)
            worklist = []
            for c in range(NCH):
                for b in range(K):
                    worklist.append((c, JLIST[b * K : (b + 1) * K], b * K, c * K + b))

            staged = None
            for widx, (c, pr, pos, gbatch) in enumerate(worklist):
                if pos == 0:
                    dens[c] = [
                        psum.tile([128, HALF], F32, tag="acc", name=f"den{c}{h}")
                        for h in range(2)
                    ]
                    nums[c] = [
                        psum.tile([128, HALF], F32, tag="acc", name=f"num{c}{h}")
                        for h in range(2)
                    ]
                head = emit_head(c, pr, pos, gbatch)
                if widx == 1:
                    emit_proj(1)
                if staged is not None:
                    emit_tail(staged)
                    if staged[2] == K * K - K and staged[0] != c:
                        emit_epilogue(staged[0])
                staged = head
            emit_tail(staged)
            emit_epilogue(staged[0])

    nc.compile()
    _dedup_ldweights(nc)
    return nc


def _block_diag_weights(w):
    """w: (G, Cg_out, Cg_in) -> lhsT layout [NCH, 128, 128] where
    lhsT[c, ci, co] = w[g, co%32, ci%32] for matching 32-blocks."""
    out = np.zeros((NCH, 128, 128), np.float32)
    for c in range(NCH):
        for g4 in range(4):
            g = c * 4 + g4
            blk = w[g]  # (Cg_out, Cg_in)
            out[c, g4 * 32 : (g4 + 1) * 32, g4 * 32 : (g4 + 1) * 32] = blk.T
    return out


_NC_CACHE = {}


def _make_in_maps(inputs):
    x = np.asarray(inputs["x"], np.float32)
    wq = np.asarray(inputs["wq"], np.float32)
    wk = np.asarray(inputs["wk"], np.float32)
    wv = np.asarray(inputs["wv"], np.float32)
    rel_emb = np.asarray(inputs["rel_emb"], np.float32)
    q_emb = np.asarray(inputs["q_emb"], np.float32)

    bf = ml_dtypes.bfloat16
    wqb = _block_diag_weights(wq).astype(bf)
    wkb = _block_diag_weights(wk).astype(bf)
    wvb = _block_diag_weights(wv).astype(bf)
    relb = np.ascontiguousarray(
        rel_emb.reshape(G, Cg, K * K).reshape(NCH, 128, K * K)
    )
    qeb = np.ascontiguousarray(q_emb.reshape(NCH, 128, 1))
    idn = np.eye(128, dtype=bf)

    xp = np.pad(x, ((0, 0), (P, P), (P, P), (0, 0)))  # (B, 60, 60, C)

    in_maps = []
    for core in range(NCORES):
        b, half = divmod(core, 2)
        sh = xp[b, HS * half : HS * half + MR]         # (32, 60, C)
        xt = np.ascontiguousarray(sh.reshape(SP, C).T).reshape(NCH, 128, SP)
        in_maps.append(
            {
                "xt": xt.astype(bf),
                "wqb": wqb,
                "wkb": wkb,
                "wvb": wvb,
                "relb": relb,
                "qeb": qeb,
                "idn": idn,
            }
        )
    return in_maps


def kernel(**inputs):
    in_maps = _make_in_maps(inputs)

    if "nc" not in _NC_CACHE:
        _NC_CACHE["nc"] = build_nc()
    nc = _NC_CACHE["nc"]

    res = run_bass_kernel_spmd(nc, in_maps, core_ids=list(range(NCORES)))

    out = np.empty((B, H, W, C), np.float32)
    for core in range(NCORES):
        b, half = divmod(core, 2)
        o = np.asarray(res.results[core]["out"]).astype(np.float32)
        o = o.reshape(C, HS, W)
        out[b, HS * half : HS * half + HS] = o.transpose(1, 2, 0)
    return out
